# revision 2
# baseline (speedup 1.0000x reference)
"""BigramLM embedding lookup as a deduplicated distributed DMA gather/scatter.

Z[b,s,:] = W[inputs[b,s],:] -- a row gather from a 256 MB table. The kernel is
SBUF-AXI-port bound: every gathered row crosses the 16 SBUF ports twice
(HBM->SBUF gather, SBUF->HBM store), ~27.2 GB/s per port.

Strategy: sort the 4096 tokens by value and give each of the 8 cores a
contiguous run of 512 sorted tokens. Within a core, duplicate tokens collapse:
only unique rows (~403 of 512) are gathered from W; each SBUF slot is then
scattered (SWDGE indirect DMA, dest-indexed) to every output position that
wants that row. This cuts gather traffic ~21% and total port traffic ~10%.

Fixed SPMD instruction stream, data-dependent work: index slots are padded
with out-of-bounds sentinels; the DMA bounds check (oob_is_err=False) makes
the hardware skip them silently.

Slot layout: unique rows sorted by multiplicity desc, slot i -> SBUF
(partition i%128, column i//128), so all duplicate-bearing slots live in the
first column(s) and occurrence tiers >= 2 are narrow early passes. Output rows
are in local sorted-token order; the host inverse-permutes at the end.
"""

from contextlib import ExitStack

import numpy as np

import concourse.bacc as bacc
import concourse.bass as bass
import concourse.mybir as mybir
from concourse.bass_utils import run_bass_kernel_spmd

VOCAB = 8192
EMB = 8192
BATCH, SEQ = 8, 512
N_CORES = 8
TOK = BATCH * SEQ // N_CORES  # 512 tokens per core
P = 128                       # SBUF partitions
GCOLS = TOK // P              # 4 slot columns (512 slots)
SENT_G = VOCAB                # > 8191 -> gather skipped
SENT_S = TOK                  # > 511  -> scatter skipped

_cache: dict = {}

# Results object of the most recent run (test.py reads exec_time_ns off it).
LAST_RESULTS = None


def _build(tier_cols):
    """tier_cols[t] = number of slot-columns occurrence-tier (t+2) spans.

    Instruction stream: 4 indirect gathers (one per slot column), then per
    gathered column c: the tier-1 scatter for c plus any dup-tier scatters
    whose column is c. All indices (gather rows, scatter dest rows) arrive in
    one [128, C] int32 blob; C = GCOLS gather cols + GCOLS tier-1 cols +
    sum(tier_cols) dup cols.
    """
    n_dup = sum(tier_cols)
    C = GCOLS + GCOLS + n_dup
    # scatter pass list: (idx_col, slot_col, ready_after_gathers)
    passes = []
    for c in range(GCOLS):
        passes.append((GCOLS + c, c, c + 1))  # tier-1 for column c
    off = 2 * GCOLS
    for t, w in enumerate(tier_cols):
        for c in range(w):
            passes.append((off, c, c + 1))
            off += 1
    # issue order: group by ready_after (ascending), dup tiers first within a
    # group so the tail is a tier-1 pass of the last column
    passes.sort(key=lambda x: (x[2], -x[0]))
    n_scatter = len(passes)

    nc = bacc.Bacc("TRN2", enable_partition_id=False, monotonic_sem_count=0)
    w = nc.dram_tensor("w", [VOCAB, EMB], mybir.dt.float32, kind="ExternalInput")
    idx = nc.dram_tensor("idx", [P, C], mybir.dt.int32, kind="ExternalInput")
    out = nc.dram_tensor("out", [TOK, EMB], mybir.dt.float32, kind="ExternalOutput")
    with (
        nc.Block() as block,
        ExitStack() as stack,
        nc.semaphore("io") as io,
        nc.semaphore("gsem") as gsem,
        nc.semaphore("ssem") as ssem,
    ):
        idx_sb = stack.enter_context(
            nc.sbuf_tensor("idx_sb", [P, C], mybir.dt.int32)
        )
        buf = stack.enter_context(
            nc.sbuf_tensor("buf", [P, GCOLS * EMB], mybir.dt.float32)
        )

        @block.gpsimd
        def _(gp):
            gp.wait_ge(io, 16)
            for c in range(GCOLS):
                gp.indirect_dma_start(
                    out=buf[:, c * EMB : (c + 1) * EMB],
                    out_offset=None,
                    in_=w[:],
                    in_offset=bass.IndirectOffsetOnAxis(
                        ap=idx_sb[:, c : c + 1], axis=0
                    ),
                    bounds_check=VOCAB - 1,
                    oob_is_err=False,
                ).then_inc(gsem, 16)
            ready = 0
            for icol, scol, need in passes:
                if need > ready:
                    gp.wait_ge(gsem, 16 * need)
                    ready = need
                gp.indirect_dma_start(
                    out=out[:],
                    out_offset=bass.IndirectOffsetOnAxis(
                        ap=idx_sb[:, icol : icol + 1], axis=0
                    ),
                    in_=buf[:, scol * EMB : (scol + 1) * EMB],
                    in_offset=None,
                    bounds_check=TOK - 1,
                    oob_is_err=False,
                ).then_inc(ssem, 16)

        @block.sync
        def _(sy):
            sy.dma_start(idx_sb[:], idx[:]).then_inc(io, 16)
            sy.wait_ge(ssem, 16 * n_scatter)

    nc.compile()
    return nc


def _plan_core(vals):
    """vals: this core's 512 sorted token values. Returns (gather_rows,
    tier1_dest, dup_tiers) where dup_tiers[t][i] is the dest row for slot i's
    occurrence t+2 (or SENT_S)."""
    uniq, first, counts = np.unique(vals, return_index=True, return_counts=True)
    order = np.argsort(-counts, kind="stable")
    uniq, first, counts = uniq[order], first[order], counts[order]
    u = len(uniq)
    g = np.full(TOK, SENT_G, np.int32)
    g[:u] = uniq
    t1 = np.full(TOK, SENT_S, np.int32)
    t1[:u] = first
    max_mult = int(counts[0]) if u else 1
    dup = []
    for t in range(2, max_mult + 1):
        n_t = int((counts >= t).sum())
        d = np.full(TOK, SENT_S, np.int32)
        d[:n_t] = first[:n_t] + (t - 1)
        dup.append(d)
    return g, t1, dup


def kernel(inputs, W):
    global LAST_RESULTS
    inputs = np.asarray(inputs)
    W = np.ascontiguousarray(np.asarray(W, dtype=np.float32))
    flat = inputs.reshape(-1).astype(np.int64)
    assert flat.shape == (N_CORES * TOK,)
    assert flat.min() >= 0 and flat.max() < VOCAB

    order = np.argsort(flat, kind="stable")
    svals = flat[order]

    plans = [
        _plan_core(svals[c * TOK : (c + 1) * TOK]) for c in range(N_CORES)
    ]
    max_tiers = max(len(p[2]) for p in plans)
    # tier_cols[t]: columns needed for occurrence tier t+2 (max over cores)
    tier_cols = []
    for t in range(max_tiers):
        n_max = max(
            int((p[2][t] != SENT_S).sum()) if t < len(p[2]) else 0
            for p in plans
        )
        tier_cols.append(max(1, -(-n_max // P)))

    key = tuple(tier_cols)
    nc = _cache.get(key)
    if nc is None:
        nc = _cache[key] = _build(tier_cols)

    in_maps = []
    for c in range(N_CORES):
        g, t1, dup = plans[c]
        # column-major slot layout: slot i -> (partition i%P, column i//P)
        cols = [g.reshape(GCOLS, P).T, t1.reshape(GCOLS, P).T]
        for t, w_t in enumerate(tier_cols):
            d = dup[t] if t < len(dup) else np.full(TOK, SENT_S, np.int32)
            cols.append(d.reshape(GCOLS, P).T[:, :w_t])
        idx2d = np.ascontiguousarray(np.concatenate(cols, axis=1), np.int32)
        in_maps.append({"w": W, "idx": idx2d})

    res = run_bass_kernel_spmd(nc, in_maps, core_ids=list(range(N_CORES)))
    LAST_RESULTS = res
    all_rows = np.concatenate(
        [res.results[c]["out"] for c in range(N_CORES)], axis=0
    )
    full = np.empty_like(all_rows)
    full[order] = all_rows
    return full.reshape(BATCH, SEQ, EMB)


# revision 3
# speedup vs baseline: 1.0003x; 1.0003x over previous
"""BigramLM embedding lookup: dedup SWDGE gather + hybrid plain/scatter store.

Value-sorted tokens split evenly over 8 cores. Each core gathers its unique
rows (~403 of 512) via indirect DMA (OOB-skipped padding), slots ordered by
occurrence-count desc so duplicate-bearing slots sit in column 0.

Stores: slot columns 0-2 go out as full 128-partition plain HWDGE stores
(dest = slot order, rows [0,384) of out). Everything else -- tier-1 of slot
column 3 and all duplicate occurrences -- goes via SWDGE indirect scatter
into a position-space region (rows [384, 896)), with OOB-skipped padding so
only real rows move. Total store traffic is exactly 512 rows/core; partial-
partition plain stores (which serialize onto DMA engines 0/1) never occur.
"""

from contextlib import ExitStack

import numpy as np

import concourse.bacc as bacc
import concourse.bass as bass
import concourse.mybir as mybir
from concourse.bass_utils import run_bass_kernel_spmd

VOCAB = 8192
EMB = 8192
BATCH, SEQ = 8, 512
N_CORES = 8
P = 128
GCOLS = 4                     # slot columns (512 slots per core)
NPLAIN = GCOLS - 1            # columns stored plain
SENT_G = VOCAB
# Cores 4 and 6 host a chronically slow SWDGE DMA engine (observed across
# every profiled run); cores 0 and 2 are occasionally affected. Skew the
# token split so the straggler engine carries proportionally less gather/
# scatter work; the fast (odd) cores absorb the difference.
TOKS = [496, 558, 496, 558, 436, 558, 436, 558]
assert sum(TOKS) == BATCH * SEQ
TOK_MAX = max(TOKS)
R = NPLAIN * P + TOK_MAX      # out rows: plain slot region + position region

_cache: dict = {}
LAST_RESULTS = None


def _build(scatter_cols):
    """scatter_cols: list of (idx_col, slot_col) scatter passes; idx blob is
    [P, GCOLS + n_passes] int32: gather cols then scatter-dest cols."""
    n_pass = len(scatter_cols)
    C = GCOLS + n_pass

    nc = bacc.Bacc("TRN2", enable_partition_id=False, monotonic_sem_count=0)
    w = nc.dram_tensor("w", [VOCAB, EMB], mybir.dt.float32, kind="ExternalInput")
    idx = nc.dram_tensor("idx", [P, C], mybir.dt.int32, kind="ExternalInput")
    out = nc.dram_tensor("out", [R, EMB], mybir.dt.float32, kind="ExternalOutput")
    with (
        nc.Block() as block,
        ExitStack() as stack,
        nc.semaphore("io") as io,
        nc.semaphore("gsem") as gsem,
        nc.semaphore("ssem") as ssem,
    ):
        idx_sb = stack.enter_context(
            nc.sbuf_tensor("idx_sb", [P, C], mybir.dt.int32)
        )
        buf = stack.enter_context(
            nc.sbuf_tensor("buf", [P, GCOLS * EMB], mybir.dt.float32)
        )

        @block.gpsimd
        def _(gp):
            gp.wait_ge(io, 16)
            for c in range(GCOLS):
                gp.indirect_dma_start(
                    out=buf[:, c * EMB : (c + 1) * EMB],
                    out_offset=None,
                    in_=w[:],
                    in_offset=bass.IndirectOffsetOnAxis(
                        ap=idx_sb[:, c : c + 1], axis=0
                    ),
                    bounds_check=VOCAB - 1,
                    oob_is_err=False,
                ).then_inc(gsem, 16)
            gp.wait_ge(io, 32)
            ready = 0
            for icol, scol in scatter_cols:
                if scol + 1 > ready:
                    gp.wait_ge(gsem, 16 * (scol + 1))
                    ready = scol + 1
                gp.indirect_dma_start(
                    out=out[:],
                    out_offset=bass.IndirectOffsetOnAxis(
                        ap=idx_sb[:, icol : icol + 1], axis=0
                    ),
                    in_=buf[:, scol * EMB : (scol + 1) * EMB],
                    in_offset=None,
                    bounds_check=R - 1,
                    oob_is_err=False,
                ).then_inc(ssem, 16)

        @block.sync
        def _(sy):
            sy.dma_start(idx_sb[:, :GCOLS], idx[:, :GCOLS]).then_inc(io, 16)
            sy.dma_start(idx_sb[:, GCOLS:], idx[:, GCOLS:]).then_inc(io, 16)
            for c in range(NPLAIN):
                sy.wait_ge(gsem, 16 * (c + 1))
                sy.dma_start(
                    out[c * P : (c + 1) * P, :],
                    buf[:, c * EMB : (c + 1) * EMB],
                    single_packet=True,
                ).then_inc(ssem, 16)
            sy.wait_ge(ssem, 16 * (NPLAIN + n_pass))

    nc.compile()
    return nc


def _plan_core(vals):
    """Bin-pack this core's unique rows onto partitions so each partition's
    SWDGE row count (1 gather + count-1 dup scatters per row) is ~equal.
    Returns slot_row[P][GCOLS] (index into uniq/first/counts, or -1) plus the
    uniq arrays. Within a partition, columns are count-desc so dup tiers
    cluster in low columns."""
    uniq, first, counts = np.unique(vals, return_index=True, return_counts=True)
    order = np.argsort(-counts, kind="stable")
    uniq, first, counts = uniq[order], first[order], counts[order]

    # partition p -> SDMA engine, per the port swizzle: even engines serve
    # partitions {4j..4j+3, 32+4j..} of 0-63, odd engines likewise for 64-127
    pe = np.empty(P, np.int64)
    for p in range(P):
        if p < 64:
            pe[p] = 2 * ((p % 32) // 4)
        else:
            pe[p] = 1 + 2 * (((p - 64) % 32) // 4)

    cost = np.zeros(P, np.int64)     # SWDGE rows on this partition
    nsl = np.zeros(P, np.int64)
    ecost = np.zeros(16, np.int64)   # SWDGE rows per engine
    slots = [[] for _ in range(P)]
    for i in range(len(uniq)):
        c = int(counts[i])
        # slot cost: 1 gather + (c-1) dup scatters + 1 tier-1 scatter if it
        # lands in the non-plain column
        add = cost + c + (nsl == GCOLS - 1)
        add[nsl >= GCOLS] = 1 << 30
        best = np.lexsort((np.arange(P), ecost[pe], add))[0]
        slots[best].append(i)
        delta = c + (1 if nsl[best] == GCOLS - 1 else 0)
        cost[best] += delta
        ecost[pe[best]] += delta
        nsl[best] += 1
    slot_row = np.full((P, GCOLS), -1, np.int64)
    for p in range(P):
        for c, i in enumerate(slots[p]):
            slot_row[p, c] = i
    return slot_row, uniq, first, counts


def kernel(inputs, W):
    global LAST_RESULTS
    inputs = np.asarray(inputs)
    W = np.ascontiguousarray(np.asarray(W, dtype=np.float32))
    flat = inputs.reshape(-1).astype(np.int64)
    assert flat.shape == (BATCH * SEQ,)
    assert flat.min() >= 0 and flat.max() < VOCAB

    order = np.argsort(flat, kind="stable")
    svals = flat[order]
    toks = list(TOKS)
    starts = np.concatenate([[0], np.cumsum(toks)])
    plans = [
        _plan_core(svals[starts[c] : starts[c + 1]]) for c in range(N_CORES)
    ]
    if any(len(p[1]) > P * GCOLS for p in plans):
        # too few duplicates for the skewed split -- fall back to an even
        # split, where uniques per core can never exceed the 512 slots
        toks = [BATCH * SEQ // N_CORES] * N_CORES
        starts = np.concatenate([[0], np.cumsum(toks)])
        plans = [
            _plan_core(svals[starts[c] : starts[c + 1]])
            for c in range(N_CORES)
        ]
    # pass_specs: (slot_col, tier) pairs needed by any core. tier 1 covers
    # first occurrences of column-NPLAIN slots (cols 0..NPLAIN-1 go plain);
    # tier t>=2 covers t-th occurrences of slots in that column.
    need = set()
    for slot_row, uniq, first, counts in plans:
        for c in range(GCOLS):
            rows = slot_row[:, c]
            real = rows >= 0
            if not real.any():
                continue
            if c == NPLAIN:
                need.add((c, 1))
            cmax = int(counts[rows[real]].max())
            for t in range(2, cmax + 1):
                need.add((c, t))
    pass_specs = sorted(need)
    scatter_cols = [
        (GCOLS + i, scol) for i, (scol, _) in enumerate(pass_specs)
    ]

    key = tuple(pass_specs)
    nc = _cache.get(key)
    if nc is None:
        nc = _cache[key] = _build(scatter_cols)

    in_maps = []
    pos_maps = []
    for cidx in range(N_CORES):
        slot_row, uniq, first, counts = plans[cidx]
        gcols = np.full((P, GCOLS), SENT_G, np.int32)
        real = slot_row >= 0
        gcols[real] = uniq[slot_row[real]]
        cols = [gcols]
        pos = np.full(R, -1, np.int64)
        for c in range(NPLAIN):  # plain region: out row c*P+p = slot (p,c)
            rc = slot_row[:, c]
            m = rc >= 0
            pos[c * P + np.nonzero(m)[0]] = first[rc[m]]
        pp = pos[NPLAIN * P :]
        for scol, t in pass_specs:
            d = np.full(P, R, np.int32)
            rows = slot_row[:, scol]
            if t == 1:
                m = rows >= 0
                d[m] = NPLAIN * P + first[rows[m]]
                pp[first[rows[m]]] = first[rows[m]]
            else:
                m = (rows >= 0) & (counts[rows.clip(0)] >= t)
                d[m] = NPLAIN * P + first[rows[m]] + (t - 1)
                pp[first[rows[m]] + t - 1] = first[rows[m]] + t - 1
            cols.append(d[:, None])
        pos[NPLAIN * P :] = pp
        idx2d = np.ascontiguousarray(np.concatenate(cols, axis=1), np.int32)
        in_maps.append({"w": W, "idx": idx2d})
        pos_maps.append(pos)

    res = run_bass_kernel_spmd(nc, in_maps, core_ids=list(range(N_CORES)))
    LAST_RESULTS = res

    full = np.empty((BATCH * SEQ, EMB), np.float32)
    for c in range(N_CORES):
        pos = pos_maps[c]
        valid = pos >= 0
        dst = order[pos[valid] + starts[c]]
        full[dst] = res.results[c]["out"][valid]
    return full.reshape(BATCH, SEQ, EMB)


# revision 4
# speedup vs baseline: 1.1666x; 1.1663x over previous
"""BigramLM embedding lookup: dedup SWDGE gather + hybrid plain/scatter store.

Value-sorted tokens split evenly over 8 cores. Each core gathers its unique
rows (~403 of 512) via indirect DMA (OOB-skipped padding), slots ordered by
occurrence-count desc so duplicate-bearing slots sit in column 0.

Stores: slot columns 0-2 go out as full 128-partition plain HWDGE stores
(dest = slot order, rows [0,384) of out). Everything else -- tier-1 of slot
column 3 and all duplicate occurrences -- goes via SWDGE indirect scatter
into a position-space region (rows [384, 896)), with OOB-skipped padding so
only real rows move. Total store traffic is exactly 512 rows/core; partial-
partition plain stores (which serialize onto DMA engines 0/1) never occur.
"""

from contextlib import ExitStack

import numpy as np

import concourse.bacc as bacc
import concourse.bass as bass
import concourse.mybir as mybir
from concourse.bass_utils import run_bass_kernel_spmd

VOCAB = 8192
EMB = 8192
BATCH, SEQ = 8, 512
N_CORES = 8
P = 128
GCOLS = 4                     # slot columns (512 slots per core)
NPLAIN = GCOLS - 1            # columns stored plain
SENT_G = VOCAB
# Cores 4 and 6 host a chronically slow SWDGE DMA engine (observed across
# every profiled run); cores 0 and 2 are occasionally affected; odd cores
# almost never straggle. Core pairs (2i, 2i+1) share an HBM stack, so each
# pair's token total is kept equal (1024) while the even core of each pair
# carries less gather/scatter work.
TOKS = [486, 538, 486, 538, 466, 558, 466, 558]
assert sum(TOKS) == BATCH * SEQ
TOK_MAX = max(TOKS)
R = NPLAIN * P + TOK_MAX      # out rows: plain slot region + position region

_cache: dict = {}
LAST_RESULTS = None


def _build(scatter_cols):
    """scatter_cols: list of (idx_col, slot_col) scatter passes; idx blob is
    [P, GCOLS + n_passes] int32: gather cols then scatter-dest cols."""
    n_pass = len(scatter_cols)
    C = GCOLS + n_pass

    nc = bacc.Bacc("TRN2", enable_partition_id=False, monotonic_sem_count=0)
    w = nc.dram_tensor("w", [VOCAB, EMB], mybir.dt.float32, kind="ExternalInput")
    idx = nc.dram_tensor("idx", [P, C], mybir.dt.int32, kind="ExternalInput")
    out = nc.dram_tensor("out", [R, EMB], mybir.dt.float32, kind="ExternalOutput")
    with (
        nc.Block() as block,
        ExitStack() as stack,
        nc.semaphore("io") as io,
        nc.semaphore("gsem") as gsem,
        nc.semaphore("ssem") as ssem,
    ):
        idx_sb = stack.enter_context(
            nc.sbuf_tensor("idx_sb", [P, C], mybir.dt.int32)
        )
        buf = stack.enter_context(
            nc.sbuf_tensor("buf", [P, GCOLS * EMB], mybir.dt.float32)
        )

        @block.gpsimd
        def _(gp):
            gp.wait_ge(io, 16)
            for c in range(GCOLS):
                gp.indirect_dma_start(
                    out=buf[:, c * EMB : (c + 1) * EMB],
                    out_offset=None,
                    in_=w[:],
                    in_offset=bass.IndirectOffsetOnAxis(
                        ap=idx_sb[:, c : c + 1], axis=0
                    ),
                    bounds_check=VOCAB - 1,
                    oob_is_err=False,
                ).then_inc(gsem, 16)
            gp.wait_ge(io, 32)
            ready = 0
            for icol, scol in scatter_cols:
                if scol + 1 > ready:
                    gp.wait_ge(gsem, 16 * (scol + 1))
                    ready = scol + 1
                gp.indirect_dma_start(
                    out=out[:],
                    out_offset=bass.IndirectOffsetOnAxis(
                        ap=idx_sb[:, icol : icol + 1], axis=0
                    ),
                    in_=buf[:, scol * EMB : (scol + 1) * EMB],
                    in_offset=None,
                    bounds_check=R - 1,
                    oob_is_err=False,
                ).then_inc(ssem, 16)

        @block.sync
        def _(sy):
            sy.dma_start(idx_sb[:, :GCOLS], idx[:, :GCOLS]).then_inc(io, 16)
            sy.dma_start(idx_sb[:, GCOLS:], idx[:, GCOLS:]).then_inc(io, 16)
            for c in range(NPLAIN):
                sy.wait_ge(gsem, 16 * (c + 1))
                sy.dma_start(
                    out[c * P : (c + 1) * P, :],
                    buf[:, c * EMB : (c + 1) * EMB],
                    single_packet=True,
                ).then_inc(ssem, 16)
            sy.wait_ge(ssem, 16 * (NPLAIN + n_pass))

    nc.compile()
    return nc


def _plan_core(vals):
    """Bin-pack this core's unique rows onto partitions so each partition's
    SWDGE row count (1 gather + count-1 dup scatters per row) is ~equal.
    Returns slot_row[P][GCOLS] (index into uniq/first/counts, or -1) plus the
    uniq arrays. Within a partition, columns are count-desc so dup tiers
    cluster in low columns."""
    uniq, first, counts = np.unique(vals, return_index=True, return_counts=True)
    order = np.argsort(-counts, kind="stable")
    uniq, first, counts = uniq[order], first[order], counts[order]

    # partition p -> SDMA engine, per the port swizzle: even engines serve
    # partitions {4j..4j+3, 32+4j..} of 0-63, odd engines likewise for 64-127
    pe = np.empty(P, np.int64)
    for p in range(P):
        if p < 64:
            pe[p] = 2 * ((p % 32) // 4)
        else:
            pe[p] = 1 + 2 * (((p - 64) % 32) // 4)

    cost = np.zeros(P, np.int64)     # SWDGE rows on this partition
    nsl = np.zeros(P, np.int64)
    ecost = np.zeros(16, np.int64)   # SWDGE rows per engine
    slots = [[] for _ in range(P)]
    for i in range(len(uniq)):
        c = int(counts[i])
        # slot cost: 1 gather + (c-1) dup scatters + 1 tier-1 scatter if it
        # lands in the non-plain column
        add = cost + c + (nsl == GCOLS - 1)
        add[nsl >= GCOLS] = 1 << 30
        best = np.lexsort((np.arange(P), ecost[pe], add))[0]
        slots[best].append(i)
        delta = c + (1 if nsl[best] == GCOLS - 1 else 0)
        cost[best] += delta
        ecost[pe[best]] += delta
        nsl[best] += 1
    slot_row = np.full((P, GCOLS), -1, np.int64)
    for p in range(P):
        for c, i in enumerate(slots[p]):
            slot_row[p, c] = i
    return slot_row, uniq, first, counts


def kernel(inputs, W):
    global LAST_RESULTS
    inputs = np.asarray(inputs)
    W = np.ascontiguousarray(np.asarray(W, dtype=np.float32))
    flat = inputs.reshape(-1).astype(np.int64)
    assert flat.shape == (BATCH * SEQ,)
    assert flat.min() >= 0 and flat.max() < VOCAB

    order = np.argsort(flat, kind="stable")
    svals = flat[order]
    toks = list(TOKS)
    starts = np.concatenate([[0], np.cumsum(toks)])
    plans = [
        _plan_core(svals[starts[c] : starts[c + 1]]) for c in range(N_CORES)
    ]
    if any(len(p[1]) > P * GCOLS for p in plans):
        # too few duplicates for the skewed split -- fall back to an even
        # split, where uniques per core can never exceed the 512 slots
        toks = [BATCH * SEQ // N_CORES] * N_CORES
        starts = np.concatenate([[0], np.cumsum(toks)])
        plans = [
            _plan_core(svals[starts[c] : starts[c + 1]])
            for c in range(N_CORES)
        ]
    # pass_specs: (slot_col, tier) pairs needed by any core. tier 1 covers
    # first occurrences of column-NPLAIN slots (cols 0..NPLAIN-1 go plain);
    # tier t>=2 covers t-th occurrences of slots in that column.
    need = set()
    for slot_row, uniq, first, counts in plans:
        for c in range(GCOLS):
            rows = slot_row[:, c]
            real = rows >= 0
            if not real.any():
                continue
            if c == NPLAIN:
                need.add((c, 1))
            cmax = int(counts[rows[real]].max())
            for t in range(2, cmax + 1):
                need.add((c, t))
    pass_specs = sorted(need)
    scatter_cols = [
        (GCOLS + i, scol) for i, (scol, _) in enumerate(pass_specs)
    ]

    key = tuple(pass_specs)
    nc = _cache.get(key)
    if nc is None:
        nc = _cache[key] = _build(scatter_cols)

    in_maps = []
    pos_maps = []
    for cidx in range(N_CORES):
        slot_row, uniq, first, counts = plans[cidx]
        gcols = np.full((P, GCOLS), SENT_G, np.int32)
        real = slot_row >= 0
        gcols[real] = uniq[slot_row[real]]
        cols = [gcols]
        pos = np.full(R, -1, np.int64)
        for c in range(NPLAIN):  # plain region: out row c*P+p = slot (p,c)
            rc = slot_row[:, c]
            m = rc >= 0
            pos[c * P + np.nonzero(m)[0]] = first[rc[m]]
        pp = pos[NPLAIN * P :]
        for scol, t in pass_specs:
            d = np.full(P, R, np.int32)
            rows = slot_row[:, scol]
            if t == 1:
                m = rows >= 0
                d[m] = NPLAIN * P + first[rows[m]]
                pp[first[rows[m]]] = first[rows[m]]
            else:
                m = (rows >= 0) & (counts[rows.clip(0)] >= t)
                d[m] = NPLAIN * P + first[rows[m]] + (t - 1)
                pp[first[rows[m]] + t - 1] = first[rows[m]] + t - 1
            cols.append(d[:, None])
        pos[NPLAIN * P :] = pp
        idx2d = np.ascontiguousarray(np.concatenate(cols, axis=1), np.int32)
        in_maps.append({"w": W, "idx": idx2d})
        pos_maps.append(pos)

    res = run_bass_kernel_spmd(nc, in_maps, core_ids=list(range(N_CORES)))
    LAST_RESULTS = res

    full = np.empty((BATCH * SEQ, EMB), np.float32)
    for c in range(N_CORES):
        pos = pos_maps[c]
        valid = pos >= 0
        dst = order[pos[valid] + starts[c]]
        full[dst] = res.results[c]["out"][valid]
    return full.reshape(BATCH, SEQ, EMB)


# revision 5
# speedup vs baseline: 1.1827x; 1.0138x over previous
"""BigramLM embedding lookup: dedup SWDGE gather + hybrid plain/scatter store.

Value-sorted tokens split evenly over 8 cores. Each core gathers its unique
rows (~403 of 512) via indirect DMA (OOB-skipped padding), slots ordered by
occurrence-count desc so duplicate-bearing slots sit in column 0.

Stores: slot columns 0-2 go out as full 128-partition plain HWDGE stores
(dest = slot order, rows [0,384) of out). Everything else -- tier-1 of slot
column 3 and all duplicate occurrences -- goes via SWDGE indirect scatter
into a position-space region (rows [384, 896)), with OOB-skipped padding so
only real rows move. Total store traffic is exactly 512 rows/core; partial-
partition plain stores (which serialize onto DMA engines 0/1) never occur.
"""

from contextlib import ExitStack

import numpy as np

import concourse.bacc as bacc
import concourse.bass as bass
import concourse.mybir as mybir
from concourse.bass_utils import run_bass_kernel_spmd

VOCAB = 8192
EMB = 8192
BATCH, SEQ = 8, 512
N_CORES = 8
P = 128
GCOLS = 4                     # slot columns (512 slots per core)
NPLAIN = GCOLS - 1            # columns stored plain
SENT_G = VOCAB
# Cores 4 and 6 host a chronically slow SWDGE DMA engine (observed across
# every profiled run); cores 0 and 2 are occasionally affected; odd cores
# almost never straggle. Core pairs (2i, 2i+1) share an HBM stack, so each
# pair's token total is kept equal (1024) while the even core of each pair
# carries less gather/scatter work.
TOKS = [466, 558, 466, 558, 466, 558, 466, 558]
assert sum(TOKS) == BATCH * SEQ
TOK_MAX = max(TOKS)
R = NPLAIN * P + TOK_MAX      # out rows: plain slot region + position region

_cache: dict = {}
LAST_RESULTS = None


def _build(scatter_cols):
    """scatter_cols: list of (idx_col, slot_col) scatter passes; idx blob is
    [P, GCOLS + n_passes] int32: gather cols then scatter-dest cols."""
    n_pass = len(scatter_cols)
    C = GCOLS + n_pass

    nc = bacc.Bacc("TRN2", enable_partition_id=False, monotonic_sem_count=0)
    w = nc.dram_tensor("w", [VOCAB, EMB], mybir.dt.float32, kind="ExternalInput")
    idx = nc.dram_tensor("idx", [P, C], mybir.dt.int32, kind="ExternalInput")
    out = nc.dram_tensor("out", [R, EMB], mybir.dt.float32, kind="ExternalOutput")
    with (
        nc.Block() as block,
        ExitStack() as stack,
        nc.semaphore("io") as io,
        nc.semaphore("gsem") as gsem,
        nc.semaphore("ssem") as ssem,
    ):
        idx_sb = stack.enter_context(
            nc.sbuf_tensor("idx_sb", [P, C], mybir.dt.int32)
        )
        buf = stack.enter_context(
            nc.sbuf_tensor("buf", [P, GCOLS * EMB], mybir.dt.float32)
        )

        @block.gpsimd
        def _(gp):
            gp.wait_ge(io, 16)
            for c in range(GCOLS):
                gp.indirect_dma_start(
                    out=buf[:, c * EMB : (c + 1) * EMB],
                    out_offset=None,
                    in_=w[:],
                    in_offset=bass.IndirectOffsetOnAxis(
                        ap=idx_sb[:, c : c + 1], axis=0
                    ),
                    bounds_check=VOCAB - 1,
                    oob_is_err=False,
                ).then_inc(gsem, 16)
            gp.wait_ge(io, 32)
            ready = 0
            for icol, scol in scatter_cols:
                if scol + 1 > ready:
                    gp.wait_ge(gsem, 16 * (scol + 1))
                    ready = scol + 1
                gp.indirect_dma_start(
                    out=out[:],
                    out_offset=bass.IndirectOffsetOnAxis(
                        ap=idx_sb[:, icol : icol + 1], axis=0
                    ),
                    in_=buf[:, scol * EMB : (scol + 1) * EMB],
                    in_offset=None,
                    bounds_check=R - 1,
                    oob_is_err=False,
                ).then_inc(ssem, 16)

        @block.sync
        def _(sy):
            sy.dma_start(idx_sb[:, :GCOLS], idx[:, :GCOLS]).then_inc(io, 16)
            sy.dma_start(idx_sb[:, GCOLS:], idx[:, GCOLS:]).then_inc(io, 16)
            for c in range(NPLAIN):
                sy.wait_ge(gsem, 16 * (c + 1))
                sy.dma_start(
                    out[c * P : (c + 1) * P, :],
                    buf[:, c * EMB : (c + 1) * EMB],
                    single_packet=True,
                ).then_inc(ssem, 16)
            sy.wait_ge(ssem, 16 * (NPLAIN + n_pass))

    nc.compile()
    return nc


def _plan_core(vals):
    """Bin-pack this core's unique rows onto partitions so each partition's
    SWDGE row count (1 gather + count-1 dup scatters per row) is ~equal.
    Returns slot_row[P][GCOLS] (index into uniq/first/counts, or -1) plus the
    uniq arrays. Within a partition, columns are count-desc so dup tiers
    cluster in low columns."""
    uniq, first, counts = np.unique(vals, return_index=True, return_counts=True)
    order = np.argsort(-counts, kind="stable")
    uniq, first, counts = uniq[order], first[order], counts[order]

    # partition p -> SDMA engine, per the port swizzle: even engines serve
    # partitions {4j..4j+3, 32+4j..} of 0-63, odd engines likewise for 64-127
    pe = np.empty(P, np.int64)
    for p in range(P):
        if p < 64:
            pe[p] = 2 * ((p % 32) // 4)
        else:
            pe[p] = 1 + 2 * (((p - 64) % 32) // 4)

    cost = np.zeros(P, np.int64)     # SWDGE rows on this partition
    nsl = np.zeros(P, np.int64)
    ecost = np.zeros(16, np.int64)   # SWDGE rows per engine
    slots = [[] for _ in range(P)]
    for i in range(len(uniq)):
        c = int(counts[i])
        # slot cost: 1 gather + (c-1) dup scatters + 1 tier-1 scatter if it
        # lands in the non-plain column
        add = cost + c + (nsl == GCOLS - 1)
        add[nsl >= GCOLS] = 1 << 30
        best = np.lexsort((np.arange(P), ecost[pe], add))[0]
        slots[best].append(i)
        delta = c + (1 if nsl[best] == GCOLS - 1 else 0)
        cost[best] += delta
        ecost[pe[best]] += delta
        nsl[best] += 1
    slot_row = np.full((P, GCOLS), -1, np.int64)
    for p in range(P):
        for c, i in enumerate(slots[p]):
            slot_row[p, c] = i
    return slot_row, uniq, first, counts


def kernel(inputs, W):
    global LAST_RESULTS
    inputs = np.asarray(inputs)
    W = np.ascontiguousarray(np.asarray(W, dtype=np.float32))
    flat = inputs.reshape(-1).astype(np.int64)
    assert flat.shape == (BATCH * SEQ,)
    assert flat.min() >= 0 and flat.max() < VOCAB

    order = np.argsort(flat, kind="stable")
    svals = flat[order]
    toks = list(TOKS)
    starts = np.concatenate([[0], np.cumsum(toks)])
    plans = [
        _plan_core(svals[starts[c] : starts[c + 1]]) for c in range(N_CORES)
    ]
    if any(len(p[1]) > P * GCOLS for p in plans):
        # too few duplicates for the skewed split -- fall back to an even
        # split, where uniques per core can never exceed the 512 slots
        toks = [BATCH * SEQ // N_CORES] * N_CORES
        starts = np.concatenate([[0], np.cumsum(toks)])
        plans = [
            _plan_core(svals[starts[c] : starts[c + 1]])
            for c in range(N_CORES)
        ]
    # pass_specs: (slot_col, tier) pairs needed by any core. tier 1 covers
    # first occurrences of column-NPLAIN slots (cols 0..NPLAIN-1 go plain);
    # tier t>=2 covers t-th occurrences of slots in that column.
    need = set()
    for slot_row, uniq, first, counts in plans:
        for c in range(GCOLS):
            rows = slot_row[:, c]
            real = rows >= 0
            if not real.any():
                continue
            if c == NPLAIN:
                need.add((c, 1))
            cmax = int(counts[rows[real]].max())
            for t in range(2, cmax + 1):
                need.add((c, t))
    pass_specs = sorted(need)
    scatter_cols = [
        (GCOLS + i, scol) for i, (scol, _) in enumerate(pass_specs)
    ]

    key = tuple(pass_specs)
    nc = _cache.get(key)
    if nc is None:
        nc = _cache[key] = _build(scatter_cols)

    in_maps = []
    pos_maps = []
    for cidx in range(N_CORES):
        slot_row, uniq, first, counts = plans[cidx]
        gcols = np.full((P, GCOLS), SENT_G, np.int32)
        real = slot_row >= 0
        gcols[real] = uniq[slot_row[real]]
        cols = [gcols]
        pos = np.full(R, -1, np.int64)
        for c in range(NPLAIN):  # plain region: out row c*P+p = slot (p,c)
            rc = slot_row[:, c]
            m = rc >= 0
            pos[c * P + np.nonzero(m)[0]] = first[rc[m]]
        pp = pos[NPLAIN * P :]
        for scol, t in pass_specs:
            d = np.full(P, R, np.int32)
            rows = slot_row[:, scol]
            if t == 1:
                m = rows >= 0
                d[m] = NPLAIN * P + first[rows[m]]
                pp[first[rows[m]]] = first[rows[m]]
            else:
                m = (rows >= 0) & (counts[rows.clip(0)] >= t)
                d[m] = NPLAIN * P + first[rows[m]] + (t - 1)
                pp[first[rows[m]] + t - 1] = first[rows[m]] + t - 1
            cols.append(d[:, None])
        pos[NPLAIN * P :] = pp
        idx2d = np.ascontiguousarray(np.concatenate(cols, axis=1), np.int32)
        in_maps.append({"w": W, "idx": idx2d})
        pos_maps.append(pos)

    res = run_bass_kernel_spmd(nc, in_maps, core_ids=list(range(N_CORES)))
    LAST_RESULTS = res

    full = np.empty((BATCH * SEQ, EMB), np.float32)
    for c in range(N_CORES):
        pos = pos_maps[c]
        valid = pos >= 0
        dst = order[pos[valid] + starts[c]]
        full[dst] = res.results[c]["out"][valid]
    return full.reshape(BATCH, SEQ, EMB)


# revision 6
# speedup vs baseline: 1.1938x; 1.0093x over previous
"""BigramLM embedding lookup: dedup SWDGE gather + hybrid plain/scatter store.

Value-sorted tokens split evenly over 8 cores. Each core gathers its unique
rows (~403 of 512) via indirect DMA (OOB-skipped padding), slots ordered by
occurrence-count desc so duplicate-bearing slots sit in column 0.

Stores: slot columns 0-2 go out as full 128-partition plain HWDGE stores
(dest = slot order, rows [0,384) of out). Everything else -- tier-1 of slot
column 3 and all duplicate occurrences -- goes via SWDGE indirect scatter
into a position-space region (rows [384, 896)), with OOB-skipped padding so
only real rows move. Total store traffic is exactly 512 rows/core; partial-
partition plain stores (which serialize onto DMA engines 0/1) never occur.
"""

from contextlib import ExitStack

import numpy as np

import concourse.bacc as bacc
import concourse.bass as bass
import concourse.mybir as mybir
from concourse.bass_utils import run_bass_kernel_spmd

VOCAB = 8192
EMB = 8192
BATCH, SEQ = 8, 512
N_CORES = 8
P = 128
GCOLS = 4                     # slot columns (512 slots per core)
NPLAIN = GCOLS - 1            # columns stored plain
SENT_G = VOCAB
# Cores 4 and 6 host a chronically slow SWDGE DMA engine (observed across
# every profiled run); cores 0 and 2 are occasionally affected; odd cores
# almost never straggle. Core pairs (2i, 2i+1) share an HBM stack, so each
# pair's token total is kept equal (1024) while the even core of each pair
# carries less gather/scatter work.
TOKS = [466, 558, 466, 558, 466, 558, 466, 558]
assert sum(TOKS) == BATCH * SEQ
TOK_MAX = max(TOKS)
R = NPLAIN * P + TOK_MAX      # out rows: plain slot region + position region

_cache: dict = {}
LAST_RESULTS = None


def _build(scatter_cols):
    """scatter_cols: list of (idx_col, slot_col) scatter passes; idx blob is
    [P, GCOLS + n_passes] int32: gather cols then scatter-dest cols."""
    n_pass = len(scatter_cols)
    C = GCOLS + n_pass

    nc = bacc.Bacc("TRN2", enable_partition_id=False, monotonic_sem_count=0)
    w = nc.dram_tensor("w", [VOCAB, EMB], mybir.dt.float32, kind="ExternalInput")
    idx = nc.dram_tensor("idx", [P, C], mybir.dt.int32, kind="ExternalInput")
    out = nc.dram_tensor("out", [R, EMB], mybir.dt.float32, kind="ExternalOutput")
    with (
        nc.Block() as block,
        ExitStack() as stack,
        nc.semaphore("io") as io,
        nc.semaphore("gsem") as gsem,
        nc.semaphore("ssem") as ssem,
    ):
        idx_sb = stack.enter_context(
            nc.sbuf_tensor("idx_sb", [P, C], mybir.dt.int32)
        )
        buf = stack.enter_context(
            nc.sbuf_tensor("buf", [P, GCOLS * EMB], mybir.dt.float32)
        )

        H = EMB // 2

        @block.gpsimd
        def _(gp):
            # half-row (16 KB) passes via element_offset: stores/scatters
            # release after half a gather column, halving all drain tails
            gp.wait_ge(io, 16)
            for c in range(GCOLS):
                for h in range(2):
                    gp.indirect_dma_start(
                        out=buf[:, c * EMB + h * H : c * EMB + (h + 1) * H],
                        out_offset=None,
                        in_=w[:],
                        in_offset=bass.IndirectOffsetOnAxis(
                            ap=idx_sb[:, c : c + 1], axis=0
                        ),
                        element_offset=h * H,
                        bounds_check=VOCAB - 1,
                        oob_is_err=False,
                    ).then_inc(gsem, 16)
            gp.wait_ge(io, 32)
            ready = 0
            for icol, scol in scatter_cols:
                for h in range(2):
                    need = 2 * scol + 1 + h
                    if need > ready:
                        gp.wait_ge(gsem, 16 * need)
                        ready = need
                    gp.indirect_dma_start(
                        out=out[:],
                        out_offset=bass.IndirectOffsetOnAxis(
                            ap=idx_sb[:, icol : icol + 1], axis=0
                        ),
                        in_=buf[:, scol * EMB + h * H : scol * EMB + (h + 1) * H],
                        in_offset=None,
                        element_offset=h * H,
                        bounds_check=R - 1,
                        oob_is_err=False,
                    ).then_inc(ssem, 16)

        @block.sync
        def _(sy):
            sy.dma_start(idx_sb[:, :GCOLS], idx[:, :GCOLS]).then_inc(io, 16)
            sy.dma_start(idx_sb[:, GCOLS:], idx[:, GCOLS:]).then_inc(io, 16)
            for c in range(NPLAIN):
                for h in range(2):
                    sy.wait_ge(gsem, 16 * (2 * c + 1 + h))
                    sy.dma_start(
                        out[c * P : (c + 1) * P, h * H : (h + 1) * H],
                        buf[:, c * EMB + h * H : c * EMB + (h + 1) * H],
                        single_packet=True,
                    ).then_inc(ssem, 16)
            sy.wait_ge(ssem, 16 * 2 * (NPLAIN + n_pass))

    nc.compile()
    return nc


def _plan_core(vals):
    """Bin-pack this core's unique rows onto partitions so each partition's
    SWDGE row count (1 gather + count-1 dup scatters per row) is ~equal.
    Returns slot_row[P][GCOLS] (index into uniq/first/counts, or -1) plus the
    uniq arrays. Within a partition, columns are count-desc so dup tiers
    cluster in low columns."""
    uniq, first, counts = np.unique(vals, return_index=True, return_counts=True)
    order = np.argsort(-counts, kind="stable")
    uniq, first, counts = uniq[order], first[order], counts[order]

    # partition p -> SDMA engine, per the port swizzle: even engines serve
    # partitions {4j..4j+3, 32+4j..} of 0-63, odd engines likewise for 64-127
    pe = np.empty(P, np.int64)
    for p in range(P):
        if p < 64:
            pe[p] = 2 * ((p % 32) // 4)
        else:
            pe[p] = 1 + 2 * (((p - 64) % 32) // 4)

    cost = np.zeros(P, np.int64)     # SWDGE rows on this partition
    nsl = np.zeros(P, np.int64)
    ecost = np.zeros(16, np.int64)   # SWDGE rows per engine
    slots = [[] for _ in range(P)]
    for i in range(len(uniq)):
        c = int(counts[i])
        # slot cost: 1 gather + (c-1) dup scatters + 1 tier-1 scatter if it
        # lands in the non-plain column
        add = cost + c + (nsl == GCOLS - 1)
        add[nsl >= GCOLS] = 1 << 30
        best = np.lexsort((np.arange(P), ecost[pe], add))[0]
        slots[best].append(i)
        delta = c + (1 if nsl[best] == GCOLS - 1 else 0)
        cost[best] += delta
        ecost[pe[best]] += delta
        nsl[best] += 1
    slot_row = np.full((P, GCOLS), -1, np.int64)
    for p in range(P):
        for c, i in enumerate(slots[p]):
            slot_row[p, c] = i
    return slot_row, uniq, first, counts


def kernel(inputs, W):
    global LAST_RESULTS
    inputs = np.asarray(inputs)
    W = np.ascontiguousarray(np.asarray(W, dtype=np.float32))
    flat = inputs.reshape(-1).astype(np.int64)
    assert flat.shape == (BATCH * SEQ,)
    assert flat.min() >= 0 and flat.max() < VOCAB

    order = np.argsort(flat, kind="stable")
    svals = flat[order]
    toks = list(TOKS)
    starts = np.concatenate([[0], np.cumsum(toks)])
    plans = [
        _plan_core(svals[starts[c] : starts[c + 1]]) for c in range(N_CORES)
    ]
    if any(len(p[1]) > P * GCOLS for p in plans):
        # too few duplicates for the skewed split -- fall back to an even
        # split, where uniques per core can never exceed the 512 slots
        toks = [BATCH * SEQ // N_CORES] * N_CORES
        starts = np.concatenate([[0], np.cumsum(toks)])
        plans = [
            _plan_core(svals[starts[c] : starts[c + 1]])
            for c in range(N_CORES)
        ]
    # pass_specs: (slot_col, tier) pairs needed by any core. tier 1 covers
    # first occurrences of column-NPLAIN slots (cols 0..NPLAIN-1 go plain);
    # tier t>=2 covers t-th occurrences of slots in that column.
    need = set()
    for slot_row, uniq, first, counts in plans:
        for c in range(GCOLS):
            rows = slot_row[:, c]
            real = rows >= 0
            if not real.any():
                continue
            if c == NPLAIN:
                need.add((c, 1))
            cmax = int(counts[rows[real]].max())
            for t in range(2, cmax + 1):
                need.add((c, t))
    pass_specs = sorted(need)
    scatter_cols = [
        (GCOLS + i, scol) for i, (scol, _) in enumerate(pass_specs)
    ]

    key = tuple(pass_specs)
    nc = _cache.get(key)
    if nc is None:
        nc = _cache[key] = _build(scatter_cols)

    in_maps = []
    pos_maps = []
    for cidx in range(N_CORES):
        slot_row, uniq, first, counts = plans[cidx]
        gcols = np.full((P, GCOLS), SENT_G, np.int32)
        real = slot_row >= 0
        gcols[real] = uniq[slot_row[real]]
        cols = [gcols]
        pos = np.full(R, -1, np.int64)
        for c in range(NPLAIN):  # plain region: out row c*P+p = slot (p,c)
            rc = slot_row[:, c]
            m = rc >= 0
            pos[c * P + np.nonzero(m)[0]] = first[rc[m]]
        pp = pos[NPLAIN * P :]
        for scol, t in pass_specs:
            d = np.full(P, R, np.int32)
            rows = slot_row[:, scol]
            if t == 1:
                m = rows >= 0
                d[m] = NPLAIN * P + first[rows[m]]
                pp[first[rows[m]]] = first[rows[m]]
            else:
                m = (rows >= 0) & (counts[rows.clip(0)] >= t)
                d[m] = NPLAIN * P + first[rows[m]] + (t - 1)
                pp[first[rows[m]] + t - 1] = first[rows[m]] + t - 1
            cols.append(d[:, None])
        pos[NPLAIN * P :] = pp
        idx2d = np.ascontiguousarray(np.concatenate(cols, axis=1), np.int32)
        in_maps.append({"w": W, "idx": idx2d})
        pos_maps.append(pos)

    res = run_bass_kernel_spmd(nc, in_maps, core_ids=list(range(N_CORES)))
    LAST_RESULTS = res

    full = np.empty((BATCH * SEQ, EMB), np.float32)
    for c in range(N_CORES):
        pos = pos_maps[c]
        valid = pos >= 0
        dst = order[pos[valid] + starts[c]]
        full[dst] = res.results[c]["out"][valid]
    return full.reshape(BATCH, SEQ, EMB)
